# revision 1
# baseline (speedup 1.0000x reference)
"""Trainium2 Bass kernel for nn_MultiHeadAttention_63814624084186.

Reference computation (per batch sample b, fully independent across b):
  x: [512, 4096]  (C channels x N=64*64 pixels)
  qkv = w_qkv @ x            -> q,k,v each [512, 4096] (8 heads x 64 dims)
  scores = (q_h @ k_h^T)/8   -> [64, 64] per head   (channel-attention)
  attn = softmax(scores, -1)
  out_h = attn_h @ v_h       -> [64, 4096]
  y = w_out @ out + b_out    -> [512, 4096]
  y = groupnorm(y over all C,N) * gamma + beta

Sharding: pure data-parallel over batch: 16 samples / 8 cores = 2 per core.

Design notes:
  - q/k/v GEMMs run in float32r (tf32-class precision at bf16-like speed
    for N=512).  x and w_q/w_k/w_v are DMA'd straight into float32r tiles
    (PE rounds on read; verified on HW).
  - phase 1+2 are n-blocked (8 blocks of 512 pixels): per block we DMA an
    x block (one DMA per channel chunk), compute qT/kT blocks ([N,512]
    layout via GEMM "transpose": lhsT = x block), accumulate scores into
    4 persistent PSUM banks, and compute v for the block.
  - scores/attn@v/out-proj run in bf16 (error contribution ~3e-3).
  - GroupNorm: bn_stats per PSUM tile (bias folded into the cross-
    partition combine), cross-partition reduce via ones-matmul.
  - The two batches are emitted interleaved:
    A(0) B(0) A(1) tail(0) B(1) tail(1), where A = blocked qkv+scores,
    B = softmax/attn@v/out-proj/bn_stats, tail = stat combine+apply+store.
    This hides batch 0's epilogue fully under batch 1's compute and keeps
    the PE queue free of stat matmuls between batches.
  - Weights arrive host-prearranged as [128, KC, C] so weight DMAs are
    contiguous per partition (few descriptors, fast issue).
"""

import numpy as np
from contextlib import ExitStack

import concourse.bass as bass
import concourse.tile as tile
from concourse import bacc, mybir
from concourse.bass_utils import run_bass_kernel_spmd

F32 = mybir.dt.float32
F32R = mybir.dt.float32r
F16 = mybir.dt.float16
BF16 = mybir.dt.bfloat16
AX = mybir.AxisListType
ALU = mybir.AluOpType
ACTF = mybir.ActivationFunctionType

B = 16          # global batch
C = 512         # channels
N = 4096        # pixels (64*64)
HW_SIDE = 64
NCORES = 8
PB = B // NCORES  # batches per core
P = 128
KC = C // P     # 4 channel chunks
NB = 8          # n blocks of 512
NBI = 4         # 128-chunks per n block
NS = N // 512   # 8 pixel chunks of 512
NHP = 4         # head pairs
EPS = 1e-5


def build_nc():
    nc = bacc.Bacc("TRN2", target_bir_lowering=False, debug=False,
                   num_devices=NCORES)

    x_d = nc.declare_dram_parameter("x", [PB, NB, P, KC * 512], F16, isOutput=False)
    wq_d = nc.declare_dram_parameter("wq", [P, KC, C], F16, isOutput=False)
    wk_d = nc.declare_dram_parameter("wk", [P, KC, C], F16, isOutput=False)
    wv_d = nc.declare_dram_parameter("wv", [P, KC, C], F16, isOutput=False)
    wo_d = nc.declare_dram_parameter("wo", [P, KC, C], F16, isOutput=False)
    bias_d = nc.declare_dram_parameter("bvec", [P, KC], F32, isOutput=False)
    gamma_d = nc.declare_dram_parameter("gamma", [P, KC], F32, isOutput=False)
    beta_d = nc.declare_dram_parameter("beta", [P, KC], F32, isOutput=False)
    out_d = nc.declare_dram_parameter("out", [PB, C, N], F16, isOutput=True)

    with tile.TileContext(nc) as tc, ExitStack() as ctx:
        consts = ctx.enter_context(tc.tile_pool(name="consts", bufs=1))
        xpool = ctx.enter_context(tc.tile_pool(name="xpool", bufs=3))
        qkpool = ctx.enter_context(tc.tile_pool(name="qkpool", bufs=4))
        vpool = ctx.enter_context(tc.tile_pool(name="vpool", bufs=2))
        w2pool = ctx.enter_context(tc.tile_pool(name="w2pool", bufs=2))
        ypool = ctx.enter_context(tc.tile_pool(name="ypool", bufs=2))
        attn = ctx.enter_context(tc.tile_pool(name="attn", bufs=8))
        attnt = ctx.enter_context(tc.tile_pool(name="attnt", bufs=4))
        stats = ctx.enter_context(tc.tile_pool(name="stats", bufs=2))
        psmm = ctx.enter_context(tc.tile_pool(name="psmm", bufs=4, space="PSUM"))
        pssc = ctx.enter_context(tc.tile_pool(name="pssc", bufs=4, space="PSUM"))

        # ---- prefetch first x blocks before weights (lead-in) ----
        # ---- interleave weight and x-block loads for minimal lead-in ----
        def load_w(dram):
            t = consts.tile([P, KC, C], F16, tag=f"w_{dram.name}")
            nc.sync.dma_start(out=t, in_=dram[:, :, :])
            return t

        prefetched_x = {}

        def prefetch_x(j):
            xt = xpool.tile([P, KC, 512], F16, tag="xblk", name=f"x_0_{j}")
            nc.sync.dma_start(
                out=xt, in_=x_d[0, j].rearrange("p (k n) -> p k n", k=KC))
            prefetched_x[j] = xt

        wq_sb = load_w(wq_d)
        prefetch_x(0)
        wk_sb = load_w(wk_d)
        prefetch_x(1)
        wv_sb = load_w(wv_d)
        prefetch_x(2)
        wo_sb = load_w(wo_d)

        bias_sb = consts.tile([P, KC], F32, tag="bias")
        nc.gpsimd.dma_start(out=bias_sb, in_=bias_d[:, :])
        gamma_sb = consts.tile([P, KC], F32, tag="gamma")
        nc.gpsimd.dma_start(out=gamma_sb, in_=gamma_d[:, :])
        beta_sb = consts.tile([P, KC], F32, tag="beta")
        nc.gpsimd.dma_start(out=beta_sb, in_=beta_d[:, :])

        eps_sb = consts.tile([1, 1], F32, tag="eps")
        nc.vector.memset(eps_sb, EPS)
        ones_col = consts.tile([P, 1], F32, tag="ones_col")
        nc.vector.memset(ones_col, 1.0)
        ones_row = consts.tile([1, P], F32, tag="ones_row")
        nc.vector.memset(ones_row, 1.0)

        # per-batch state carried between emission stages
        st_v = {}
        st_sc = {}
        st_y = {}
        st_stats = {}

        def emit_A_setup(b):
            v_sb = vpool.tile([P, NHP, N], F16, tag="v", name=f"v_{b}")
            sc_ps = [pssc.tile([P, 64], F32, tag="pssc", name=f"sc_{b}_{hp}")
                     for hp in range(NHP)]
            st_v[b] = v_sb
            st_sc[b] = sc_ps

        def emit_A_blocks(b, blocks):
            """n-blocked qT/kT GEMMs, score accumulation, v GEMM."""
            v_sb = st_v[b]
            sc_ps = st_sc[b]
            for j in blocks:
                if b == 0 and j in prefetched_x:
                    x_blk = prefetched_x[j]
                else:
                    x_blk = xpool.tile([P, KC, 512], F16, tag="xblk",
                                       name=f"x_{b}_{j}")
                    nc.sync.dma_start(
                        out=x_blk,
                        in_=x_d[b, j].rearrange("p (k n) -> p k n", k=KC))

                qT_blk = qkpool.tile([P, NBI, C], F16, tag="qk",
                                     name=f"qT_{b}_{j}")
                kT_blk = qkpool.tile([P, NBI, C], F16, tag="qk",
                                     name=f"kT_{b}_{j}")
                for dst, w in ((qT_blk, wq_sb), (kT_blk, wk_sb)):
                    for i in range(NBI):
                        ps = psmm.tile([P, C], F32, tag="psmm")
                        for k in range(KC):
                            nc.tensor.matmul(
                                ps,
                                lhsT=x_blk[:, k, i * P:(i + 1) * P],
                                rhs=w[:, k, :],
                                start=(k == 0), stop=(k == KC - 1))
                        nc.scalar.copy(out=dst[:, i, :], in_=ps)

                for hp in range(NHP):
                    cl0 = slice(hp * P, hp * P + 64)
                    cl1 = slice(hp * P + 64, (hp + 1) * P)
                    for i in range(NBI):
                        st_flag = (j == 0 and i == 0)
                        sp_flag = (j == NB - 1 and i == NBI - 1)
                        # two heads' [64,64] scores run concurrently in
                        # separate PE column groups (col tiling); head A in
                        # psum rows 0:64, head B in rows 64:128, same columns
                        nc.tensor.matmul(
                            sc_ps[hp][0:64, :],
                            lhsT=qT_blk[:, i, cl0],
                            rhs=kT_blk[:, i, cl0],
                            start=st_flag, stop=sp_flag,
                            skip_group_check=True)
                        nc.tensor.matmul(
                            sc_ps[hp][64:P, :],
                            lhsT=qT_blk[:, i, cl1],
                            rhs=kT_blk[:, i, cl1],
                            start=st_flag, stop=sp_flag,
                            skip_group_check=True)

                for hp in range(NHP):
                    cl = slice(hp * P, (hp + 1) * P)
                    ps = psmm.tile([P, 512], F32, tag="psmm")
                    for k in range(KC):
                        nc.tensor.matmul(
                            ps,
                            lhsT=wv_sb[:, k, cl],
                            rhs=x_blk[:, k, :],
                            start=(k == 0), stop=(k == KC - 1))
                    nc.vector.tensor_copy(
                        out=v_sb[:, hp, j * 512:(j + 1) * 512], in_=ps)

        st_ao = {}
        st_at = {}

        def emit_softmax(b):
            """softmax on the accumulated score blocks (all pairs batched)."""
            sc_ps = st_sc[b]
            a_all = attn.tile([P, NHP, 64], F32, tag="a_all")
            for hp in range(NHP):
                nc.vector.tensor_copy(out=a_all[:, hp, :], in_=sc_ps[hp])
            mx = attn.tile([P, NHP, 1], F32, tag="mx4")
            nc.vector.reduce_max(out=mx, in_=a_all, axis=AX.X)
            d_all = attn.tile([P, NHP, 64], F32, tag="d_all")
            nc.vector.tensor_tensor(d_all, a_all,
                                    mx.to_broadcast([P, NHP, 64]), ALU.subtract)
            e_all = attn.tile([P, NHP, 64], F32, tag="e_all")
            nc.scalar.activation(out=e_all, in_=d_all, func=ACTF.Exp,
                                 bias=0.0, scale=0.125)
            sm = attn.tile([P, NHP, 1], F32, tag="sm4")
            nc.vector.reduce_sum(out=sm, in_=e_all, axis=AX.X)
            rs = attn.tile([P, NHP, 1], F32, tag="rs4")
            nc.vector.reciprocal(out=rs, in_=sm)
            a_mm = attn.tile([P, NHP, 64], F16, tag="amm4")
            nc.vector.tensor_tensor(a_mm, e_all,
                                    rs.to_broadcast([P, NHP, 64]), ALU.mult)
            attnT_tiles = []
            for hp in range(NHP):
                at = attnt.tile([P, P], F16, tag="attnT", name=f"at_{b}_{hp}")
                nc.gpsimd.memset(at, 0.0)
                attnT_tiles.append((at, a_mm[:, hp, :]))
            st_at[b] = attnT_tiles

        def emit_W2(b):
            """fold attn into the out-projection: W2 = blockdiag(A)^T @ woT."""
            attnT_tiles = st_at[b]
            w2 = w2pool.tile([P, KC, C], F16, tag="w2", name=f"w2_{b}")
            for hp in range(NHP):
                at, a_mm = attnT_tiles[hp]
                # block-diagonal attn (untransposed): out = A^T @ woT rows
                nc.vector.tensor_copy(out=at[0:64, 0:64], in_=a_mm[0:64, :])
                nc.vector.tensor_copy(out=at[64:P, 64:P], in_=a_mm[64:P, :])
                ps = psmm.tile([P, C], F32, tag="psmm")
                nc.tensor.matmul(ps, lhsT=at, rhs=wo_sb[:, hp, :],
                                 start=True, stop=True)
                if hp % 2 == 0:
                    nc.scalar.copy(out=w2[:, hp, :], in_=ps)
                else:
                    nc.vector.tensor_copy(out=w2[:, hp, :], in_=ps)
            st_ao[b] = w2

        def emit_By(b):
            """out projection (fused weights) + bn_stats."""
            w2 = st_ao[b]
            v_sb = st_v[b]
            y_lo = ypool.tile([P, 2, N], F16, tag="y", name=f"ylo_{b}")
            y_hi = ypool.tile([P, 2, N], F16, tag="y", name=f"yhi_{b}")
            st = stats.tile([P, KC, NS, 6], F32, tag="bnstats")
            mv_t = stats.tile([P, KC, 2], F32, tag="mv")
            st_y[b] = (y_lo, y_hi)
            st_stats[b] = mv_t
            for m in range(KC):
                yt = y_lo if m < 2 else y_hi
                mi = m % 2
                for ns in range(NS):
                    ps = psmm.tile([P, 512], F32, tag="psmm")
                    for k in range(KC):
                        nc.tensor.matmul(
                            ps,
                            lhsT=w2[:, k, m * P:(m + 1) * P],
                            rhs=v_sb[:, k, ns * 512:(ns + 1) * 512],
                            start=(k == 0), stop=(k == KC - 1))
                    # stats on pre-bias values (bias folded in below)
                    nc.vector.bn_stats(out=st[:, m, ns, :], in_=ps)
                    nc.scalar.add(out=yt[:, mi, ns * 512:(ns + 1) * 512],
                                  in_=ps, add=bias_sb[:, m:m + 1])
                nc.vector.bn_aggr(out=mv_t[:, m, :], in_=st[:, m])

        st_scale = {}

        def emit_tail_stats(b):
            """global mean/var combine."""
            mv = st_stats[b]
            # S[p, stat, m]: 0 = mean+bias, 1 = var, 2 = (mean+bias)^2
            s_t = stats.tile([P, 3, KC], F32, tag="s_t")
            nc.vector.tensor_add(s_t[:, 0, :], mv[:, :, 0], bias_sb)
            nc.vector.tensor_copy(out=s_t[:, 1, :], in_=mv[:, :, 1])
            nc.vector.tensor_mul(s_t[:, 2, :], s_t[:, 0, :], s_t[:, 0, :])
            pstat = psmm.tile([1, 3, KC], F32, tag="psmm")
            nc.tensor.matmul(pstat, lhsT=ones_col, rhs=s_t,
                             start=True, stop=True)
            red = stats.tile([1, 3], F32, tag="red")
            nc.vector.reduce_sum(out=red, in_=pstat, axis=AX.X)
            e3 = stats.tile([1, 3], F32, tag="e3")
            nc.vector.tensor_scalar_mul(e3, red, 1.0 / C)
            m2 = stats.tile([1, 1], F32, tag="m2")
            nc.vector.tensor_mul(m2, e3[:, 0:1], e3[:, 0:1])
            var = stats.tile([1, 1], F32, tag="var")
            nc.vector.tensor_add(var, e3[:, 1:2], e3[:, 2:3])
            nc.vector.tensor_sub(var, var, m2)
            sc2 = stats.tile([1, 2], F32, tag="sc2")
            nc.vector.tensor_copy(out=sc2[:, 0:1], in_=e3[:, 0:1])
            std = stats.tile([1, 1], F32, tag="std")
            nc.scalar.activation(out=std, in_=var, func=ACTF.Sqrt,
                                 bias=eps_sb, scale=1.0)
            nc.vector.reciprocal(out=sc2[:, 1:2], in_=std)
            bc_ps = psmm.tile([P, 2], F32, tag="psmm")
            nc.tensor.matmul(bc_ps, lhsT=ones_row, rhs=sc2,
                             start=True, stop=True)
            # s = gamma * rstd ; t = beta - mean_total * s
            s_ch = stats.tile([P, KC], F32, tag="s_ch")
            nc.vector.tensor_scalar_mul(s_ch, gamma_sb, bc_ps[:, 1:2])
            t_ch = stats.tile([P, KC], F32, tag="t_ch")
            nc.vector.tensor_scalar_mul(t_ch, s_ch, bc_ps[:, 0:1])
            nc.vector.tensor_sub(t_ch, beta_sb, t_ch)
            st_scale[b] = (s_ch, t_ch)

        def emit_tail_apply(b):
            """normalization apply + writeout."""
            y_lo, y_hi = st_y[b]
            s_ch, t_ch = st_scale[b]
            for m in range(KC):
                yt = y_lo if m < 2 else y_hi
                mi = m % 2
                for h in range(2):
                    sl = slice(h * (N // 2), (h + 1) * (N // 2))
                    if b == 0 or m % 2 == 0:
                        nc.vector.tensor_scalar(
                            out=yt[:, mi, sl], in0=yt[:, mi, sl],
                            scalar1=s_ch[:, m:m + 1], scalar2=t_ch[:, m:m + 1],
                            op0=ALU.mult, op1=ALU.add)
                    else:
                        nc.scalar.activation(
                            out=yt[:, mi, sl], in_=yt[:, mi, sl],
                            func=ACTF.Identity,
                            bias=t_ch[:, m:m + 1], scale=s_ch[:, m:m + 1])
                    nc.sync.dma_start(out=out_d[b, m * P:(m + 1) * P, sl],
                                      in_=yt[:, mi, sl])

        emit_A_setup(0)
        emit_A_blocks(0, range(NB))
        emit_softmax(0)
        emit_A_setup(1)
        emit_A_blocks(1, range(2))
        emit_W2(0)
        emit_By(0)
        emit_A_blocks(1, range(2, NB))
        emit_softmax(1)
        emit_tail_stats(0)
        emit_W2(1)
        emit_tail_apply(0)
        emit_By(1)
        emit_tail_stats(1)
        emit_tail_apply(1)

    nc.finalize()
    return nc


_NC_CACHE = {}


def _get_nc():
    if "nc" not in _NC_CACHE:
        _NC_CACHE["nc"] = build_nc()
    return _NC_CACHE["nc"]


def _prep_w(w):
    # [C_in, C_out] -> [128, KC, C_out] fp16 with c_in = k*128 + p
    return np.ascontiguousarray(
        w.reshape(KC, P, C).transpose(1, 0, 2).astype(np.float16))


def _prep_vec(v):
    # [C] -> [128, KC] with c = k*128 + p
    return np.ascontiguousarray(v.reshape(KC, P).T)


def _prep_x(x):
    # [B, C, N] -> [B, NB, P, KC*512] fp16: block j, partition p, (k, n)
    nb = np.asarray(x).shape[0]
    xr = np.asarray(x, dtype=np.float32).reshape(nb, KC, P, NB, 512)
    return np.ascontiguousarray(
        xr.transpose(0, 3, 2, 1, 4).astype(np.float16)).reshape(
        nb, NB, P, KC * 512)


def _prep_x_local(x):
    return _prep_x(x)


def _make_in_maps(x, w_qkv, w_out, b_out, gamma, beta):
    xr = _prep_x(x)
    w_qkv = np.asarray(w_qkv, dtype=np.float32)
    wq = _prep_w(np.ascontiguousarray(w_qkv[0:C].T))
    wk = _prep_w(np.ascontiguousarray(w_qkv[C:2 * C].T))
    wv = _prep_w(np.ascontiguousarray(w_qkv[2 * C:3 * C].T))
    wo = _prep_w(np.ascontiguousarray(np.asarray(w_out, dtype=np.float32).T))
    bvec = _prep_vec(np.asarray(b_out, dtype=np.float32))
    gam = _prep_vec(np.asarray(gamma, dtype=np.float32))
    bet = _prep_vec(np.asarray(beta, dtype=np.float32))
    return [
        dict(x=np.ascontiguousarray(xr[c * PB:(c + 1) * PB]),
             wq=wq, wk=wk, wv=wv, wo=wo,
             bvec=bvec, gamma=gam, beta=bet)
        for c in range(NCORES)
    ]


def _run(inputs, trace=False, trace_kwargs=None):
    nc = _get_nc()
    in_maps = _make_in_maps(**inputs)
    res = run_bass_kernel_spmd(nc, in_maps, core_ids=list(range(NCORES)),
                               trace=trace, **(trace_kwargs or {}))
    out = np.concatenate([res.results[c]["out"].astype(np.float32)
                          for c in range(NCORES)], axis=0)
    return out.reshape(B, C, HW_SIDE, HW_SIDE), res


def kernel(x, w_qkv, w_out, b_out, gamma, beta):
    inputs = dict(x=x, w_qkv=w_qkv, w_out=w_out, b_out=b_out,
                  gamma=gamma, beta=beta)
    try:
        out, _ = _run(inputs)
    except Exception:
        # transient device errors (e.g. NRT_EXEC_UNIT_UNRECOVERABLE) have
        # been observed once across many runs; one retry recovers.
        out, _ = _run(inputs)
    return out



# revision 10
# speedup vs baseline: 1.6263x; 1.6263x over previous
"""Trainium2 Bass kernel for nn_MultiHeadAttention_63814624084186.

Reference computation (per batch sample b, fully independent across b):
  x: [512, 4096]  (C channels x N=64*64 pixels)
  qkv = w_qkv @ x            -> q,k,v each [512, 4096] (8 heads x 64 dims)
  scores = (q_h @ k_h^T)/8   -> [64, 64] per head   (channel-attention)
  attn = softmax(scores, -1)
  out_h = attn_h @ v_h       -> [64, 4096]
  y = w_out @ out + b_out    -> [512, 4096]
  y = groupnorm(y over all C,N) * gamma + beta

Key algebra (this version): attention is over the CHANNEL dim, so
  scores_h = q_h k_h^T = (w_q G w_k^T)_h   with  G = x x^T  [512,512]
  y = w_out bd(A) w_v x = W_eff x          with  W_eff folded on-chip
q, k, v are never materialized.  Per-batch PE work drops from ~4.5e9
MACs (qkv + v + out-proj) to ~2.4e9 (G + y GEMM + small folds).

Sharding: pure data-parallel over batch: 16 samples / 8 cores = 2 per core.

Pipeline (PE queue order; b0/b1 are the two per-core batches):
  G0 T0 sc0 | G1 | R0 W20 T1 sc1 | y0 | R1 W21 | y1 | tails
softmax(b) runs on DVE under the next long PE phase, so the PE never
waits on it.  GroupNorm: bn_stats per psum block, bias folded into the
cross-partition combine (ones-matmul), apply + writeout overlap y1.
"""

import numpy as np
from contextlib import ExitStack

import concourse.bass as bass
import concourse.tile as tile
from concourse import bacc, mybir
from concourse.bass_utils import run_bass_kernel_spmd

F32 = mybir.dt.float32
F16 = mybir.dt.float16
AX = mybir.AxisListType
ALU = mybir.AluOpType
ACTF = mybir.ActivationFunctionType

B = 16          # global batch
C = 512         # channels
N = 4096        # pixels (64*64)
HW_SIDE = 64
NCORES = 8
PB = B // NCORES  # batches per core
P = 128
KC = C // P     # 4 channel chunks
NB = 8          # n blocks of 512 (y GEMM)
NT = 16         # xT tiles of 2 n-chunks each (G GEMM)
NS = N // 512   # 8 pixel chunks of 512
NHP = 4         # head pairs
XLOOK = 6       # xT DMA lookahead tiles
EPS = 1e-5


def build_nc():
    nc = bacc.Bacc("TRN2", target_bir_lowering=False, debug=False,
                   num_devices=NCORES)

    # xT[b, t, p, j*512+c] = x[b, c, (2t+j)*128 + p]
    xt_d = nc.declare_dram_parameter("xt", [PB, NT, P, 1024], F16, isOutput=False)
    # x[b, nb, p, k*512+n] = x[b, k*128+p, nb*512+n]
    x_d = nc.declare_dram_parameter("x", [PB, NB, P, KC * 512], F16, isOutput=False)
    wq_d = nc.declare_dram_parameter("wq", [P, KC, C], F16, isOutput=False)   # w_q^T
    wk_d = nc.declare_dram_parameter("wk", [P, KC, C], F16, isOutput=False)   # w_k^T
    wv_d = nc.declare_dram_parameter("wv", [P, KC, C], F16, isOutput=False)   # w_v
    wo_d = nc.declare_dram_parameter("wo", [P, KC, C], F16, isOutput=False)   # w_out^T
    bias_d = nc.declare_dram_parameter("bvec", [P, KC], F32, isOutput=False)
    gamma_d = nc.declare_dram_parameter("gamma", [P, KC], F32, isOutput=False)
    beta_d = nc.declare_dram_parameter("beta", [P, KC], F32, isOutput=False)
    out_d = nc.declare_dram_parameter("out", [PB, C, N], F16, isOutput=True)

    with tile.TileContext(nc) as tc, ExitStack() as ctx:
        consts = ctx.enter_context(tc.tile_pool(name="consts", bufs=1))
        xtpool = ctx.enter_context(tc.tile_pool(name="xtpool", bufs=XLOOK + 2))
        xpool = ctx.enter_context(tc.tile_pool(name="xpool", bufs=3))
        gpool = ctx.enter_context(tc.tile_pool(name="gpool", bufs=2))
        tpool = ctx.enter_context(tc.tile_pool(name="tpool", bufs=2))
        rpool = ctx.enter_context(tc.tile_pool(name="rpool", bufs=2))
        w2pool = ctx.enter_context(tc.tile_pool(name="w2pool", bufs=2))
        ypool = ctx.enter_context(tc.tile_pool(name="ypool", bufs=4))
        attn = ctx.enter_context(tc.tile_pool(name="attn", bufs=8))
        attnt = ctx.enter_context(tc.tile_pool(name="attnt", bufs=4))
        stats = ctx.enter_context(tc.tile_pool(name="stats", bufs=4))
        # psg serves both G (4 full banks) and the scores tiles: a matmul
        # start=True resets the target bank's whole per-partition row, so
        # each head-pair's score accumulator needs its own bank (partition
        # packing 0:64/64:128 within a bank is safe, free-offset packing
        # is NOT).  The pool rotation reuses G's banks once G is copied out.
        psg = ctx.enter_context(tc.tile_pool(name="psg", bufs=4, space="PSUM"))
        psmm = ctx.enter_context(tc.tile_pool(name="psmm", bufs=3, space="PSUM"))

        def load_w(dram):
            t = consts.tile([P, KC, C], F16, tag=f"w_{dram.name}")
            nc.sync.dma_start(out=t, in_=dram[:, :, :])
            return t

        xt_tiles = {}

        def fetch_xt(b, t):
            xt = xtpool.tile([P, 2, 512], F16, tag="xt", name=f"xt_{b}_{t}")
            nc.sync.dma_start(
                out=xt, in_=xt_d[b, t].rearrange("p (j c) -> p j c", j=2))
            xt_tiles[(b, t)] = xt

        # weave the (later-needed) weight loads between early xT fetches so
        # the first G matmuls aren't DMA-gated.
        for t in range(3):
            fetch_xt(0, t)
        wk_sb = load_w(wk_d)
        for t in range(3, 5):
            fetch_xt(0, t)
        wq_sb = load_w(wq_d)
        fetch_xt(0, 5)
        wo_sb = load_w(wo_d)
        wv_sb = load_w(wv_d)

        bias_sb = consts.tile([P, KC], F32, tag="bias")
        nc.gpsimd.dma_start(out=bias_sb, in_=bias_d[:, :])
        gamma_sb = consts.tile([P, KC], F32, tag="gamma")
        nc.gpsimd.dma_start(out=gamma_sb, in_=gamma_d[:, :])
        beta_sb = consts.tile([P, KC], F32, tag="beta")
        nc.gpsimd.dma_start(out=beta_sb, in_=beta_d[:, :])

        eps_sb = consts.tile([1, 1], F32, tag="eps")
        nc.vector.memset(eps_sb, EPS)
        ones_col = consts.tile([P, 1], F32, tag="ones_col")
        nc.vector.memset(ones_col, 1.0)
        ones_row = consts.tile([1, P], F32, tag="ones_row")
        nc.vector.memset(ones_row, 1.0)

        # per-batch state carried between emission stages
        st_g = {}    # G in SBUF (f16) [P, KC, C]
        st_t = {}    # T = G @ wk^T   [P, KC, C]
        st_sc = {}   # scores psum tiles (4x [P, 64], head-pair packed)
        st_at = {}   # block-diag attn tiles
        st_r = {}    # R = bd(A)^T @ wo^T
        st_w2 = {}   # W_effT = wv^T-contract @ R
        st_y = {}
        st_stats = {}
        st_scale = {}

        def emit_G(b, prefetched):
            """G = x x^T, t-outer: each xT tile is consumed then retired.
            All 4 output chunks accumulate in 4 psum banks simultaneously."""
            g_sb = gpool.tile([P, KC, C], F16, tag="g", name=f"g_{b}")
            st_g[b] = g_sb
            ps = [psg.tile([P, C], F32, tag="psg", name=f"g_{b}_{m}")
                  for m in range(KC)]
            for t in range(NT):
                tf = t + prefetched
                if tf < NT:
                    fetch_xt(b, tf)
                elif b + 1 < PB and tf - NT < NT:
                    fetch_xt(b + 1, tf - NT)
                xt = xt_tiles.pop((b, t))
                for j in range(2):
                    for m in range(KC):
                        nc.tensor.matmul(
                            ps[m],
                            lhsT=xt[:, j, m * P:(m + 1) * P],
                            rhs=xt[:, j, :],
                            start=(t == 0 and j == 0),
                            stop=(t == NT - 1 and j == 1),
                            skip_group_check=True)
            for m in range(KC):
                if m % 2 == 0:
                    nc.scalar.copy(out=g_sb[:, m, :], in_=ps[m])
                else:
                    nc.vector.tensor_copy(out=g_sb[:, m, :], in_=ps[m])

        def emit_T(b):
            """T = G @ wk^T  [c, e], m-outer single-bank accumulation."""
            g_sb = st_g[b]
            t_sb = tpool.tile([P, KC, C], F16, tag="t", name=f"t_{b}")
            st_t[b] = t_sb
            for m in range(KC):
                ps = psmm.tile([P, C], F32, tag="psmm")
                for mp in range(KC):
                    nc.tensor.matmul(
                        ps,
                        lhsT=g_sb[:, mp, m * P:(m + 1) * P],
                        rhs=wk_sb[:, mp, :],
                        start=(mp == 0), stop=(mp == KC - 1))
                nc.vector.tensor_copy(out=t_sb[:, m, :], in_=ps)

        def emit_scores(b):
            """scores_h = (w_q T)_h, two heads packed per psum tile; k-outer
            so the first matmuls only need T chunk 0."""
            t_sb = st_t[b]
            sc_ps = [psg.tile([P, 64], F32, tag="psg", name=f"sc_{b}_{hp}")
                     for hp in range(NHP)]
            st_sc[b] = sc_ps
            for k in range(KC):
                for hp in range(NHP):
                    hA, hB = 2 * hp, 2 * hp + 1
                    clA = slice(hA * 64, hA * 64 + 64)
                    clB = slice(hB * 64, hB * 64 + 64)
                    nc.tensor.matmul(
                        sc_ps[hp][0:64, :],
                        lhsT=wq_sb[:, k, clA], rhs=t_sb[:, k, clA],
                        start=(k == 0), stop=(k == KC - 1),
                        skip_group_check=True)
                    nc.tensor.matmul(
                        sc_ps[hp][64:P, :],
                        lhsT=wq_sb[:, k, clB], rhs=t_sb[:, k, clB],
                        start=(k == 0), stop=(k == KC - 1),
                        skip_group_check=True)

        def emit_softmax(b):
            """softmax over scores (all head pairs batched) -> blockdiag tiles."""
            sc_ps = st_sc[b]
            a_all = attn.tile([P, NHP, 64], F32, tag="a_all")
            for hp in range(NHP):
                nc.vector.tensor_copy(out=a_all[:, hp, :], in_=sc_ps[hp])
            mx = attn.tile([P, NHP, 1], F32, tag="mx4")
            nc.vector.reduce_max(out=mx, in_=a_all, axis=AX.X)
            d_all = attn.tile([P, NHP, 64], F32, tag="d_all")
            nc.vector.tensor_tensor(d_all, a_all,
                                    mx.to_broadcast([P, NHP, 64]), ALU.subtract)
            e_all = attn.tile([P, NHP, 64], F32, tag="e_all")
            nc.scalar.activation(out=e_all, in_=d_all, func=ACTF.Exp,
                                 bias=0.0, scale=0.125)
            sm = attn.tile([P, NHP, 1], F32, tag="sm4")
            nc.vector.reduce_sum(out=sm, in_=e_all, axis=AX.X)
            rs = attn.tile([P, NHP, 1], F32, tag="rs4")
            nc.vector.reciprocal(out=rs, in_=sm)
            a_mm = attn.tile([P, NHP, 64], F16, tag="amm4")
            nc.vector.tensor_tensor(a_mm, e_all,
                                    rs.to_broadcast([P, NHP, 64]), ALU.mult)
            bd_tiles = []
            for hp in range(NHP):
                at = attnt.tile([P, P], F16, tag="attnT", name=f"at_{b}_{hp}")
                nc.gpsimd.memset(at, 0.0)
                nc.vector.tensor_copy(out=at[0:64, 0:64], in_=a_mm[0:64, hp, :])
                nc.vector.tensor_copy(out=at[64:P, 64:P], in_=a_mm[64:P, hp, :])
                bd_tiles.append(at)
            st_at[b] = bd_tiles

        def emit_R(b):
            """R[e, o] = sum_d bd(A)[d, e] wo^T[d, o]."""
            bd_tiles = st_at[b]
            r_sb = rpool.tile([P, KC, C], F16, tag="r", name=f"r_{b}")
            st_r[b] = r_sb
            for hp in range(NHP):
                ps = psmm.tile([P, C], F32, tag="psmm")
                nc.tensor.matmul(ps, lhsT=bd_tiles[hp], rhs=wo_sb[:, hp, :],
                                 start=True, stop=True)
                if hp % 2 == 0:
                    nc.scalar.copy(out=r_sb[:, hp, :], in_=ps)
                else:
                    nc.vector.tensor_copy(out=r_sb[:, hp, :], in_=ps)

        def emit_W2(b):
            """W_effT[c, o] = sum_e wv[e, c] R[e, o], m-outer."""
            r_sb = st_r[b]
            w2 = w2pool.tile([P, KC, C], F16, tag="w2", name=f"w2_{b}")
            st_w2[b] = w2
            for m in range(KC):
                ps = psmm.tile([P, C], F32, tag="psmm")
                for ki in range(KC):
                    nc.tensor.matmul(
                        ps,
                        lhsT=wv_sb[:, ki, m * P:(m + 1) * P],
                        rhs=r_sb[:, ki, :],
                        start=(ki == 0), stop=(ki == KC - 1))
                if m % 2 == 0:
                    nc.scalar.copy(out=w2[:, m, :], in_=ps)
                else:
                    nc.vector.tensor_copy(out=w2[:, m, :], in_=ps)

        def emit_By(b):
            """y = W_eff @ x (+bias) + bn_stats, streaming x blocks."""
            w2 = st_w2[b]
            y_lo = ypool.tile([P, 2, N], F16, tag="y", name=f"ylo_{b}")
            y_hi = ypool.tile([P, 2, N], F16, tag="y", name=f"yhi_{b}")
            st = stats.tile([P, KC, NS, 6], F32, tag="bnstats")
            mv_t = stats.tile([P, KC, 2], F32, tag="mv")
            st_y[b] = (y_lo, y_hi)
            st_stats[b] = mv_t
            x_blks = {}

            def fetch_x(ns):
                xb = xpool.tile([P, KC, 512], F16, tag="xblk",
                                name=f"x_{b}_{ns}")
                nc.sync.dma_start(
                    out=xb, in_=x_d[b, ns].rearrange("p (k n) -> p k n", k=KC))
                x_blks[ns] = xb

            fetch_x(0)
            fetch_x(1)
            for ns in range(NS):
                if ns + 2 < NS:
                    fetch_x(ns + 2)
                x_blk = x_blks.pop(ns)
                for m in range(KC):
                    yt = y_lo if m < 2 else y_hi
                    mi = m % 2
                    ps = psmm.tile([P, 512], F32, tag="psmm")
                    for k in range(KC):
                        nc.tensor.matmul(
                            ps,
                            lhsT=w2[:, k, m * P:(m + 1) * P],
                            rhs=x_blk[:, k, :],
                            start=(k == 0), stop=(k == KC - 1))
                    # stats on pre-bias values (bias folded into the combine)
                    nc.vector.bn_stats(out=st[:, m, ns, :], in_=ps)
                    nc.scalar.add(out=yt[:, mi, ns * 512:(ns + 1) * 512],
                                  in_=ps, add=bias_sb[:, m:m + 1])
            for m in range(KC):
                nc.vector.bn_aggr(out=mv_t[:, m, :], in_=st[:, m])

        def emit_tail_stats(b):
            """global mean/var combine."""
            mv = st_stats[b]
            # S[p, stat, m]: 0 = mean+bias, 1 = var, 2 = (mean+bias)^2
            s_t = stats.tile([P, 3, KC], F32, tag="s_t")
            nc.vector.tensor_add(s_t[:, 0, :], mv[:, :, 0], bias_sb)
            nc.vector.tensor_copy(out=s_t[:, 1, :], in_=mv[:, :, 1])
            nc.vector.tensor_mul(s_t[:, 2, :], s_t[:, 0, :], s_t[:, 0, :])
            pstat = psmm.tile([1, 3, KC], F32, tag="psmm")
            nc.tensor.matmul(pstat, lhsT=ones_col, rhs=s_t,
                             start=True, stop=True)
            red = stats.tile([1, 3], F32, tag="red")
            nc.vector.reduce_sum(out=red, in_=pstat, axis=AX.X)
            e3 = stats.tile([1, 3], F32, tag="e3")
            nc.vector.tensor_scalar_mul(e3, red, 1.0 / C)
            m2 = stats.tile([1, 1], F32, tag="m2")
            nc.vector.tensor_mul(m2, e3[:, 0:1], e3[:, 0:1])
            var = stats.tile([1, 1], F32, tag="var")
            nc.vector.tensor_add(var, e3[:, 1:2], e3[:, 2:3])
            nc.vector.tensor_sub(var, var, m2)
            sc2 = stats.tile([1, 2], F32, tag="sc2")
            nc.vector.tensor_copy(out=sc2[:, 0:1], in_=e3[:, 0:1])
            std = stats.tile([1, 1], F32, tag="std")
            nc.scalar.activation(out=std, in_=var, func=ACTF.Sqrt,
                                 bias=eps_sb, scale=1.0)
            nc.vector.reciprocal(out=sc2[:, 1:2], in_=std)
            bc_ps = psmm.tile([P, 2], F32, tag="psmm")
            nc.tensor.matmul(bc_ps, lhsT=ones_row, rhs=sc2,
                             start=True, stop=True)
            # s = gamma * rstd ; t = beta - mean_total * s
            s_ch = stats.tile([P, KC], F32, tag="s_ch")
            nc.vector.tensor_scalar_mul(s_ch, gamma_sb, bc_ps[:, 1:2])
            t_ch = stats.tile([P, KC], F32, tag="t_ch")
            nc.vector.tensor_scalar_mul(t_ch, s_ch, bc_ps[:, 0:1])
            nc.vector.tensor_sub(t_ch, beta_sb, t_ch)
            st_scale[b] = (s_ch, t_ch)

        def emit_tail_apply(b):
            """normalization apply + writeout."""
            y_lo, y_hi = st_y[b]
            s_ch, t_ch = st_scale[b]
            for m in range(KC):
                yt = y_lo if m < 2 else y_hi
                mi = m % 2
                for h in range(2):
                    sl = slice(h * (N // 2), (h + 1) * (N // 2))
                    if m % 2 == 0:
                        nc.vector.tensor_scalar(
                            out=yt[:, mi, sl], in0=yt[:, mi, sl],
                            scalar1=s_ch[:, m:m + 1], scalar2=t_ch[:, m:m + 1],
                            op0=ALU.mult, op1=ALU.add)
                    else:
                        nc.scalar.activation(
                            out=yt[:, mi, sl], in_=yt[:, mi, sl],
                            func=ACTF.Identity,
                            bias=t_ch[:, m:m + 1], scale=s_ch[:, m:m + 1])
                    nc.sync.dma_start(out=out_d[b, m * P:(m + 1) * P, sl],
                                      in_=yt[:, mi, sl])

        # ---- emission schedule (PE queue order is emission order) ----
        emit_G(0, prefetched=6)
        emit_T(0)
        emit_scores(0)
        emit_softmax(0)       # DVE, overlaps G1 on PE
        emit_G(1, prefetched=6)
        emit_R(0)
        emit_W2(0)
        emit_T(1)
        emit_scores(1)
        emit_softmax(1)       # DVE, overlaps y0 on PE
        emit_By(0)
        emit_R(1)
        emit_W2(1)
        emit_tail_stats(0)
        emit_By(1)
        emit_tail_apply(0)    # DVE/ACT + DMA, overlaps y1 on PE
        emit_tail_stats(1)
        emit_tail_apply(1)

    nc.finalize()
    return nc


_NC_CACHE = {}


def _get_nc():
    if "nc" not in _NC_CACHE:
        _NC_CACHE["nc"] = build_nc()
    return _NC_CACHE["nc"]


def _prep_w(w):
    # [C_in, C_out] -> [128, KC, C_out] fp16 with c_in = k*128 + p
    return np.ascontiguousarray(
        w.reshape(KC, P, C).transpose(1, 0, 2).astype(np.float16))


def _prep_vec(v):
    # [C] -> [128, KC] with c = k*128 + p
    return np.ascontiguousarray(v.reshape(KC, P).T)


def _prep_x(x):
    # [B, C, N] -> [B, NB, P, KC*512] fp16: block j, partition p, (k, n)
    nb = x.shape[0]
    xr = x.reshape(nb, KC, P, NB, 512)
    return np.ascontiguousarray(
        xr.transpose(0, 3, 2, 1, 4).astype(np.float16)).reshape(
        nb, NB, P, KC * 512)


def _prep_xt(x):
    # [B, C, N] -> [B, NT, P, 2*512] fp16: xt[b,t,p,j*512+c] = x[b,c,(2t+j)*128+p]
    nb = x.shape[0]
    xr = x.reshape(nb, C, NT, 2, P)           # [b, c, t, j, p]
    return np.ascontiguousarray(
        xr.transpose(0, 2, 4, 3, 1).astype(np.float16)).reshape(
        nb, NT, P, 1024)


def _make_in_maps(x, w_qkv, w_out, b_out, gamma, beta):
    x = np.asarray(x, dtype=np.float32).reshape(B, C, N)
    xr = _prep_x(x)
    xtr = _prep_xt(x)
    w_qkv = np.asarray(w_qkv, dtype=np.float32)
    wq = _prep_w(np.ascontiguousarray(w_qkv[0:C].T))
    wk = _prep_w(np.ascontiguousarray(w_qkv[C:2 * C].T))
    wv = _prep_w(np.ascontiguousarray(w_qkv[2 * C:3 * C]))
    wo = _prep_w(np.ascontiguousarray(np.asarray(w_out, dtype=np.float32).T))
    bvec = _prep_vec(np.asarray(b_out, dtype=np.float32))
    gam = _prep_vec(np.asarray(gamma, dtype=np.float32))
    bet = _prep_vec(np.asarray(beta, dtype=np.float32))
    return [
        dict(x=np.ascontiguousarray(xr[c * PB:(c + 1) * PB]),
             xt=np.ascontiguousarray(xtr[c * PB:(c + 1) * PB]),
             wq=wq, wk=wk, wv=wv, wo=wo,
             bvec=bvec, gamma=gam, beta=bet)
        for c in range(NCORES)
    ]


def _run(inputs, trace=False, trace_kwargs=None):
    nc = _get_nc()
    in_maps = _make_in_maps(**inputs)
    res = run_bass_kernel_spmd(nc, in_maps, core_ids=list(range(NCORES)),
                               trace=trace, **(trace_kwargs or {}))
    out = np.concatenate([res.results[c]["out"].astype(np.float32)
                          for c in range(NCORES)], axis=0)
    return out.reshape(B, C, HW_SIDE, HW_SIDE), res


def kernel(x, w_qkv, w_out, b_out, gamma, beta):
    inputs = dict(x=x, w_qkv=w_qkv, w_out=w_out, b_out=b_out,
                  gamma=gamma, beta=beta)
    try:
        out, _ = _run(inputs)
    except Exception:
        # transient device errors (e.g. NRT_EXEC_UNIT_UNRECOVERABLE) have
        # been observed once across many runs; one retry recovers.
        out, _ = _run(inputs)
    return out


# revision 16
# speedup vs baseline: 1.7409x; 1.0705x over previous
"""Trainium2 Bass kernel for nn_MultiHeadAttention_63814624084186.

Reference computation (per batch sample b, fully independent across b):
  x: [512, 4096]  (C channels x N=64*64 pixels)
  qkv = w_qkv @ x            -> q,k,v each [512, 4096] (8 heads x 64 dims)
  scores = (q_h @ k_h^T)/8   -> [64, 64] per head   (channel-attention)
  attn = softmax(scores, -1)
  out_h = attn_h @ v_h       -> [64, 4096]
  y = w_out @ out + b_out    -> [512, 4096]
  y = groupnorm(y over all C,N) * gamma + beta

Key algebra (this version): attention is over the CHANNEL dim, so
  scores_h = q_h k_h^T = (w_q G w_k^T)_h   with  G = x x^T  [512,512]
  y = w_out bd(A) w_v x = W_eff x          with  W_eff folded on-chip
q, k, v are never materialized.  Per-batch PE work drops from ~4.5e9
MACs (qkv + v + out-proj) to ~2.4e9 (G + y GEMM + small folds).

Sharding: pure data-parallel over batch: 16 samples / 8 cores = 2 per core.

Pipeline (PE queue order; b0/b1 are the two per-core batches):
  G0 T0 sc0 | G1 | R0 W20 T1 sc1 | y0 | R1 W21 | y1 | tails
softmax(b) runs on DVE under the next long PE phase, so the PE never
waits on it.  GroupNorm: bn_stats per psum block, bias folded into the
cross-partition combine (ones-matmul), apply + writeout overlap y1.
"""

import numpy as np
from contextlib import ExitStack

import concourse.bass as bass
import concourse.tile as tile
from concourse import bacc, mybir
from concourse.bass_utils import run_bass_kernel_spmd
from concourse.masks import make_identity

F32 = mybir.dt.float32
F16 = mybir.dt.float16
AX = mybir.AxisListType
ALU = mybir.AluOpType
ACTF = mybir.ActivationFunctionType

B = 16          # global batch
C = 512         # channels
N = 4096        # pixels (64*64)
HW_SIDE = 64
NCORES = 8
PB = B // NCORES  # batches per core
P = 128
KC = C // P     # 4 channel chunks
NB = 8          # n blocks of 512 (y GEMM)
NT = 16         # xT tiles of 2 n-chunks each (G GEMM)
NS = N // 512   # 8 pixel chunks of 512
NHP = 4         # head pairs
XLOOK = 6       # xT DMA lookahead tiles
EPS = 1e-5


def build_nc():
    nc = bacc.Bacc("TRN2", target_bir_lowering=False, debug=False,
                   num_devices=NCORES)

    # xT[b, t, p, j*512+c] = x[b, c, (2t+j)*128 + p]
    xt_d = nc.declare_dram_parameter("xt", [PB, NT, P, 1024], F16, isOutput=False)
    # x[b, nb, p, k*512+n] = x[b, k*128+p, nb*512+n]
    x_d = nc.declare_dram_parameter("x", [PB, NB, P, KC * 512], F16, isOutput=False)
    wq_d = nc.declare_dram_parameter("wq", [P, KC, C], F16, isOutput=False)   # w_q^T
    wk_d = nc.declare_dram_parameter("wk", [P, KC, C], F16, isOutput=False)   # w_k^T
    wv_d = nc.declare_dram_parameter("wv", [P, KC, C], F16, isOutput=False)   # w_v
    wo_d = nc.declare_dram_parameter("wo", [P, KC, C], F16, isOutput=False)   # w_out^T
    bias_d = nc.declare_dram_parameter("bvec", [P, KC], F32, isOutput=False)
    gamma_d = nc.declare_dram_parameter("gamma", [P, KC], F32, isOutput=False)
    beta_d = nc.declare_dram_parameter("beta", [P, KC], F32, isOutput=False)
    out_d = nc.declare_dram_parameter("out", [PB, C, N], F16, isOutput=True)

    with tile.TileContext(nc) as tc, ExitStack() as ctx:
        consts = ctx.enter_context(tc.tile_pool(name="consts", bufs=1))
        xtpool = ctx.enter_context(tc.tile_pool(name="xtpool", bufs=XLOOK + 2))
        xpool = ctx.enter_context(tc.tile_pool(name="xpool", bufs=3))
        gpool = ctx.enter_context(tc.tile_pool(name="gpool", bufs=2))
        tpool = ctx.enter_context(tc.tile_pool(name="tpool", bufs=2))
        rpool = ctx.enter_context(tc.tile_pool(name="rpool", bufs=2))
        w2pool = ctx.enter_context(tc.tile_pool(name="w2pool", bufs=2))
        ypool = ctx.enter_context(tc.tile_pool(name="ypool", bufs=4))
        attn = ctx.enter_context(tc.tile_pool(name="attn", bufs=8))
        attnt = ctx.enter_context(tc.tile_pool(name="attnt", bufs=4))
        stats = ctx.enter_context(tc.tile_pool(name="stats", bufs=4))
        # psg serves both G (4 full banks) and the scores tiles: a matmul
        # start=True resets the target bank's whole per-partition row, so
        # each head-pair's score accumulator needs its own bank (partition
        # packing 0:64/64:128 within a bank is safe, free-offset packing
        # is NOT).  The pool rotation reuses G's banks once G is copied out.
        psg = ctx.enter_context(tc.tile_pool(name="psg", bufs=4, space="PSUM"))
        psmm = ctx.enter_context(tc.tile_pool(name="psmm", bufs=4, space="PSUM"))

        def load_w(dram):
            t = consts.tile([P, KC, C], F16, tag=f"w_{dram.name}")
            nc.sync.dma_start(out=t, in_=dram[:, :, :])
            return t

        xt_tiles = {}

        def fetch_xt(b, t):
            xt = xtpool.tile([P, 2, 512], F16, tag="xt", name=f"xt_{b}_{t}")
            nc.sync.dma_start(
                out=xt, in_=xt_d[b, t].rearrange("p (j c) -> p j c", j=2))
            xt_tiles[(b, t)] = xt

        # weave the (later-needed) weight loads between early xT fetches so
        # the first G matmuls aren't DMA-gated.
        for t in range(3):
            fetch_xt(0, t)
        wk_sb = load_w(wk_d)
        for t in range(3, 5):
            fetch_xt(0, t)
        wq_sb = load_w(wq_d)
        fetch_xt(0, 5)
        wo_sb = load_w(wo_d)
        wv_sb = load_w(wv_d)

        bias_sb = consts.tile([P, KC], F32, tag="bias")
        nc.gpsimd.dma_start(out=bias_sb, in_=bias_d[:, :])
        gamma_sb = consts.tile([P, KC], F32, tag="gamma")
        nc.gpsimd.dma_start(out=gamma_sb, in_=gamma_d[:, :])
        beta_sb = consts.tile([P, KC], F32, tag="beta")
        nc.gpsimd.dma_start(out=beta_sb, in_=beta_d[:, :])

        ident_sb = consts.tile([P, P], F16, tag="ident")
        make_identity(nc, ident_sb)
        eps_sb = consts.tile([1, 1], F32, tag="eps")
        nc.vector.memset(eps_sb, EPS)
        ones_col = consts.tile([P, 1], F32, tag="ones_col")
        nc.vector.memset(ones_col, 1.0)
        ones_row = consts.tile([1, P], F32, tag="ones_row")
        nc.vector.memset(ones_row, 1.0)

        # per-batch state carried between emission stages
        st_g = {}    # G in SBUF (f16) [P, KC, C]
        st_t = {}    # T = G @ wk^T   [P, KC, C]
        st_sc = {}   # scores psum tiles (4x [P, 64], head-pair packed)
        st_at = {}   # block-diag attn tiles
        st_r = {}    # R = bd(A)^T @ wo^T
        st_w2 = {}   # W_effT = wv^T-contract @ R
        st_y = {}
        st_stats = {}
        st_scale = {}

        def emit_G(b, prefetched):
            """G = x x^T, t-outer: each xT tile is consumed then retired.
            Only the upper block-triangle is computed (rhs = cols >= m*128);
            the 6 lower [128,128] blocks are PE-transposed from the upper
            copies.  All 4 chunk accumulators live in 4 psum banks."""
            g_sb = gpool.tile([P, KC, C], F16, tag="g", name=f"g_{b}")
            st_g[b] = g_sb
            ps = [psg.tile([P, C - m * P], F32, tag="psg", name=f"g_{b}_{m}")
                  for m in range(KC)]
            for t in range(NT):
                tf = t + prefetched
                if tf < NT:
                    fetch_xt(b, tf)
                elif b + 1 < PB and tf - NT < NT:
                    fetch_xt(b + 1, tf - NT)
                xt = xt_tiles.pop((b, t))
                for j in range(2):
                    for m in range(KC):
                        nc.tensor.matmul(
                            ps[m],
                            lhsT=xt[:, j, m * P:(m + 1) * P],
                            rhs=xt[:, j, m * P:],
                            start=(t == 0 and j == 0),
                            stop=(t == NT - 1 and j == 1),
                            skip_group_check=True)
            for m in range(KC):
                if m % 2 == 0:
                    nc.scalar.copy(out=g_sb[:, m, m * P:], in_=ps[m])
                else:
                    nc.vector.tensor_copy(out=g_sb[:, m, m * P:], in_=ps[m])
            # lower blocks (m, mp<m) = transpose(upper block (mp, m)),
            # ordered so T chunk 0's operands are ready first
            for m, mp in ((1, 0), (2, 0), (3, 0), (2, 1), (3, 1), (3, 2)):
                pst = psmm.tile([P, P], F16, tag="psmm")
                nc.tensor.transpose(
                    pst, g_sb[:, mp, m * P:(m + 1) * P], ident_sb)
                if (m + mp) % 2 == 0:
                    nc.vector.tensor_copy(
                        out=g_sb[:, m, mp * P:(mp + 1) * P], in_=pst)
                else:
                    nc.scalar.copy(
                        out=g_sb[:, m, mp * P:(mp + 1) * P], in_=pst)

        def emit_T(b):
            """T = G @ wk^T  [c, e], m-outer single-bank accumulation."""
            g_sb = st_g[b]
            t_sb = tpool.tile([P, KC, C], F16, tag="t", name=f"t_{b}")
            st_t[b] = t_sb
            for m in range(KC):
                ps = psmm.tile([P, C], F32, tag="psmm")
                for mp in range(KC):
                    nc.tensor.matmul(
                        ps,
                        lhsT=g_sb[:, mp, m * P:(m + 1) * P],
                        rhs=wk_sb[:, mp, :],
                        start=(mp == 0), stop=(mp == KC - 1))
                nc.vector.tensor_copy(out=t_sb[:, m, :], in_=ps)

        def emit_scores(b):
            """scores_h = (w_q T)_h, two heads packed per psum tile; k-outer
            so the first matmuls only need T chunk 0."""
            t_sb = st_t[b]
            sc_ps = [psg.tile([P, 64], F32, tag="psg", name=f"sc_{b}_{hp}")
                     for hp in range(NHP)]
            st_sc[b] = sc_ps
            for k in range(KC):
                for hp in range(NHP):
                    hA, hB = 2 * hp, 2 * hp + 1
                    clA = slice(hA * 64, hA * 64 + 64)
                    clB = slice(hB * 64, hB * 64 + 64)
                    nc.tensor.matmul(
                        sc_ps[hp][0:64, :],
                        lhsT=wq_sb[:, k, clA], rhs=t_sb[:, k, clA],
                        start=(k == 0), stop=(k == KC - 1),
                        skip_group_check=True)
                    nc.tensor.matmul(
                        sc_ps[hp][64:P, :],
                        lhsT=wq_sb[:, k, clB], rhs=t_sb[:, k, clB],
                        start=(k == 0), stop=(k == KC - 1),
                        skip_group_check=True)

        def emit_softmax(b):
            """softmax over scores (all head pairs batched) -> blockdiag tiles."""
            sc_ps = st_sc[b]
            a_all = attn.tile([P, NHP, 64], F32, tag="a_all")
            for hp in range(NHP):
                nc.vector.tensor_copy(out=a_all[:, hp, :], in_=sc_ps[hp])
            mx = attn.tile([P, NHP, 1], F32, tag="mx4")
            nc.vector.reduce_max(out=mx, in_=a_all, axis=AX.X)
            d_all = attn.tile([P, NHP, 64], F32, tag="d_all")
            nc.vector.tensor_tensor(d_all, a_all,
                                    mx.to_broadcast([P, NHP, 64]), ALU.subtract)
            e_all = attn.tile([P, NHP, 64], F32, tag="e_all")
            nc.scalar.activation(out=e_all, in_=d_all, func=ACTF.Exp,
                                 bias=0.0, scale=0.125)
            sm = attn.tile([P, NHP, 1], F32, tag="sm4")
            nc.vector.reduce_sum(out=sm, in_=e_all, axis=AX.X)
            rs = attn.tile([P, NHP, 1], F32, tag="rs4")
            nc.vector.reciprocal(out=rs, in_=sm)
            a_mm = attn.tile([P, NHP, 64], F16, tag="amm4")
            nc.vector.tensor_tensor(a_mm, e_all,
                                    rs.to_broadcast([P, NHP, 64]), ALU.mult)
            bd_tiles = []
            for hp in range(NHP):
                at = attnt.tile([P, P], F16, tag="attnT", name=f"at_{b}_{hp}")
                nc.gpsimd.memset(at, 0.0)
                nc.vector.tensor_copy(out=at[0:64, 0:64], in_=a_mm[0:64, hp, :])
                nc.vector.tensor_copy(out=at[64:P, 64:P], in_=a_mm[64:P, hp, :])
                bd_tiles.append(at)
            st_at[b] = bd_tiles

        def emit_R(b):
            """R[e, o] = sum_d bd(A)[d, e] wo^T[d, o]."""
            bd_tiles = st_at[b]
            r_sb = rpool.tile([P, KC, C], F16, tag="r", name=f"r_{b}")
            st_r[b] = r_sb
            for hp in range(NHP):
                ps = psmm.tile([P, C], F32, tag="psmm")
                nc.tensor.matmul(ps, lhsT=bd_tiles[hp], rhs=wo_sb[:, hp, :],
                                 start=True, stop=True)
                if hp % 2 == 0:
                    nc.scalar.copy(out=r_sb[:, hp, :], in_=ps)
                else:
                    nc.vector.tensor_copy(out=r_sb[:, hp, :], in_=ps)

        def emit_W2(b):
            """W_effT[c, o] = sum_e wv[e, c] R[e, o], m-outer."""
            r_sb = st_r[b]
            w2 = w2pool.tile([P, KC, C], F16, tag="w2", name=f"w2_{b}")
            st_w2[b] = w2
            for m in range(KC):
                ps = psmm.tile([P, C], F32, tag="psmm")
                for ki in range(KC):
                    nc.tensor.matmul(
                        ps,
                        lhsT=wv_sb[:, ki, m * P:(m + 1) * P],
                        rhs=r_sb[:, ki, :],
                        start=(ki == 0), stop=(ki == KC - 1))
                if m % 2 == 0:
                    nc.scalar.copy(out=w2[:, m, :], in_=ps)
                else:
                    nc.vector.tensor_copy(out=w2[:, m, :], in_=ps)

        st_by = {}

        def emit_By_setup(b):
            y_lo = ypool.tile([P, 2, N], F16, tag="y", name=f"ylo_{b}")
            y_hi = ypool.tile([P, 2, N], F16, tag="y", name=f"yhi_{b}")
            st = stats.tile([P, KC, NS, 6], F32, tag="bnstats")
            mv_t = stats.tile([P, KC, 2], F32, tag="mv")
            st_y[b] = (y_lo, y_hi)
            st_stats[b] = mv_t
            st_by[b] = (st, {})
            for ns in range(2):
                xb = xpool.tile([P, KC, 512], F16, tag="xblk",
                                name=f"x_{b}_{ns}")
                nc.sync.dma_start(
                    out=xb, in_=x_d[b, ns].rearrange("p (k n) -> p k n", k=KC))
                st_by[b][1][ns] = xb

        def emit_By_blocks(b, blocks):
            """y = W_eff @ x (+bias) + bn_stats, streaming x blocks."""
            w2 = st_w2[b]
            y_lo, y_hi = st_y[b]
            st, x_blks = st_by[b]
            for ns in blocks:
                if ns + 2 < NS:
                    xb = xpool.tile([P, KC, 512], F16, tag="xblk",
                                    name=f"x_{b}_{ns + 2}")
                    nc.sync.dma_start(
                        out=xb,
                        in_=x_d[b, ns + 2].rearrange("p (k n) -> p k n", k=KC))
                    x_blks[ns + 2] = xb
                x_blk = x_blks.pop(ns)
                for m in range(KC):
                    yt = y_lo if m < 2 else y_hi
                    mi = m % 2
                    ps = psmm.tile([P, 512], F32, tag="psmm")
                    for k in range(KC):
                        nc.tensor.matmul(
                            ps,
                            lhsT=w2[:, k, m * P:(m + 1) * P],
                            rhs=x_blk[:, k, :],
                            start=(k == 0), stop=(k == KC - 1))
                    # stats on pre-bias values (bias folded into the combine)
                    nc.vector.bn_stats(out=st[:, m, ns, :], in_=ps)
                    nc.scalar.add(out=yt[:, mi, ns * 512:(ns + 1) * 512],
                                  in_=ps, add=bias_sb[:, m:m + 1])

        def emit_By_aggr(b):
            st, _ = st_by[b]
            mv_t = st_stats[b]
            for m in range(KC):
                nc.vector.bn_aggr(out=mv_t[:, m, :], in_=st[:, m])

        def emit_tail_stats(b):
            """global mean/var combine."""
            mv = st_stats[b]
            # S[p, stat, m]: 0 = mean+bias, 1 = var, 2 = (mean+bias)^2
            s_t = stats.tile([P, 3, KC], F32, tag="s_t")
            nc.vector.tensor_add(s_t[:, 0, :], mv[:, :, 0], bias_sb)
            nc.vector.tensor_copy(out=s_t[:, 1, :], in_=mv[:, :, 1])
            nc.vector.tensor_mul(s_t[:, 2, :], s_t[:, 0, :], s_t[:, 0, :])
            pstat = psmm.tile([1, 3, KC], F32, tag="psmm")
            nc.tensor.matmul(pstat, lhsT=ones_col, rhs=s_t,
                             start=True, stop=True)
            red = stats.tile([1, 3], F32, tag="red")
            nc.vector.reduce_sum(out=red, in_=pstat, axis=AX.X)
            e3 = stats.tile([1, 3], F32, tag="e3")
            nc.vector.tensor_scalar_mul(e3, red, 1.0 / C)
            m2 = stats.tile([1, 1], F32, tag="m2")
            nc.vector.tensor_mul(m2, e3[:, 0:1], e3[:, 0:1])
            var = stats.tile([1, 1], F32, tag="var")
            nc.vector.tensor_add(var, e3[:, 1:2], e3[:, 2:3])
            nc.vector.tensor_sub(var, var, m2)
            sc2 = stats.tile([1, 2], F32, tag="sc2")
            nc.vector.tensor_copy(out=sc2[:, 0:1], in_=e3[:, 0:1])
            std = stats.tile([1, 1], F32, tag="std")
            nc.scalar.activation(out=std, in_=var, func=ACTF.Sqrt,
                                 bias=eps_sb, scale=1.0)
            nc.vector.reciprocal(out=sc2[:, 1:2], in_=std)
            bc_ps = psmm.tile([P, 2], F32, tag="psmm")
            nc.tensor.matmul(bc_ps, lhsT=ones_row, rhs=sc2,
                             start=True, stop=True)
            # s = gamma * rstd ; t = beta - mean_total * s
            s_ch = stats.tile([P, KC], F32, tag="s_ch")
            nc.vector.tensor_scalar_mul(s_ch, gamma_sb, bc_ps[:, 1:2])
            t_ch = stats.tile([P, KC], F32, tag="t_ch")
            nc.vector.tensor_scalar_mul(t_ch, s_ch, bc_ps[:, 0:1])
            nc.vector.tensor_sub(t_ch, beta_sb, t_ch)
            st_scale[b] = (s_ch, t_ch)

        def emit_tail_apply(b):
            """normalization apply + writeout."""
            y_lo, y_hi = st_y[b]
            s_ch, t_ch = st_scale[b]
            for m in range(KC):
                yt = y_lo if m < 2 else y_hi
                mi = m % 2
                for h in range(2):
                    sl = slice(h * (N // 2), (h + 1) * (N // 2))
                    if m % 2 == 0:
                        nc.vector.tensor_scalar(
                            out=yt[:, mi, sl], in0=yt[:, mi, sl],
                            scalar1=s_ch[:, m:m + 1], scalar2=t_ch[:, m:m + 1],
                            op0=ALU.mult, op1=ALU.add)
                    else:
                        nc.scalar.activation(
                            out=yt[:, mi, sl], in_=yt[:, mi, sl],
                            func=ACTF.Identity,
                            bias=t_ch[:, m:m + 1], scale=s_ch[:, m:m + 1])
                    nc.sync.dma_start(out=out_d[b, m * P:(m + 1) * P, sl],
                                      in_=yt[:, mi, sl])

        # ---- emission schedule (PE queue order is emission order) ----
        emit_G(0, prefetched=6)
        emit_T(0)
        emit_scores(0)
        emit_softmax(0)       # DVE, overlaps G1 on PE
        emit_G(1, prefetched=6)
        emit_R(0)
        emit_W2(0)
        emit_T(1)
        emit_scores(1)
        emit_softmax(1)       # DVE, overlaps y0 on PE
        emit_By_setup(0)
        emit_By_blocks(0, range(NS))
        emit_By_aggr(0)
        emit_R(1)
        emit_W2(1)
        emit_By_setup(1)
        emit_By_blocks(1, range(2))
        emit_tail_stats(0)    # stat-combine chain hides under By1 blocks
        emit_tail_apply(0)    # DVE/ACT + DMA, overlaps y1 on PE
        emit_By_blocks(1, range(2, NS))
        emit_By_aggr(1)
        emit_tail_stats(1)
        emit_tail_apply(1)

    nc.finalize()
    return nc


_NC_CACHE = {}


def _get_nc():
    if "nc" not in _NC_CACHE:
        _NC_CACHE["nc"] = build_nc()
    return _NC_CACHE["nc"]


def _prep_w(w):
    # [C_in, C_out] -> [128, KC, C_out] fp16 with c_in = k*128 + p
    return np.ascontiguousarray(
        w.reshape(KC, P, C).transpose(1, 0, 2).astype(np.float16))


def _prep_vec(v):
    # [C] -> [128, KC] with c = k*128 + p
    return np.ascontiguousarray(v.reshape(KC, P).T)


def _prep_x(x):
    # [B, C, N] -> [B, NB, P, KC*512] fp16: block j, partition p, (k, n)
    nb = x.shape[0]
    xr = x.reshape(nb, KC, P, NB, 512)
    return np.ascontiguousarray(
        xr.transpose(0, 3, 2, 1, 4).astype(np.float16)).reshape(
        nb, NB, P, KC * 512)


def _prep_xt(x):
    # [B, C, N] -> [B, NT, P, 2*512] fp16: xt[b,t,p,j*512+c] = x[b,c,(2t+j)*128+p]
    nb = x.shape[0]
    xr = x.reshape(nb, C, NT, 2, P)           # [b, c, t, j, p]
    return np.ascontiguousarray(
        xr.transpose(0, 2, 4, 3, 1).astype(np.float16)).reshape(
        nb, NT, P, 1024)


def _make_in_maps(x, w_qkv, w_out, b_out, gamma, beta):
    x = np.asarray(x, dtype=np.float32).reshape(B, C, N)
    xr = _prep_x(x)
    xtr = _prep_xt(x)
    w_qkv = np.asarray(w_qkv, dtype=np.float32)
    wq = _prep_w(np.ascontiguousarray(w_qkv[0:C].T))
    wk = _prep_w(np.ascontiguousarray(w_qkv[C:2 * C].T))
    wv = _prep_w(np.ascontiguousarray(w_qkv[2 * C:3 * C]))
    wo = _prep_w(np.ascontiguousarray(np.asarray(w_out, dtype=np.float32).T))
    bvec = _prep_vec(np.asarray(b_out, dtype=np.float32))
    gam = _prep_vec(np.asarray(gamma, dtype=np.float32))
    bet = _prep_vec(np.asarray(beta, dtype=np.float32))
    return [
        dict(x=np.ascontiguousarray(xr[c * PB:(c + 1) * PB]),
             xt=np.ascontiguousarray(xtr[c * PB:(c + 1) * PB]),
             wq=wq, wk=wk, wv=wv, wo=wo,
             bvec=bvec, gamma=gam, beta=bet)
        for c in range(NCORES)
    ]


def _run(inputs, trace=False, trace_kwargs=None):
    nc = _get_nc()
    in_maps = _make_in_maps(**inputs)
    res = run_bass_kernel_spmd(nc, in_maps, core_ids=list(range(NCORES)),
                               trace=trace, **(trace_kwargs or {}))
    out = np.concatenate([res.results[c]["out"].astype(np.float32)
                          for c in range(NCORES)], axis=0)
    return out.reshape(B, C, HW_SIDE, HW_SIDE), res


def kernel(x, w_qkv, w_out, b_out, gamma, beta):
    inputs = dict(x=x, w_qkv=w_qkv, w_out=w_out, b_out=b_out,
                  gamma=gamma, beta=beta)
    try:
        out, _ = _run(inputs)
    except Exception:
        # transient device errors (e.g. NRT_EXEC_UNIT_UNRECOVERABLE) have
        # been observed once across many runs; one retry recovers.
        out, _ = _run(inputs)
    return out


# revision 24
# speedup vs baseline: 1.7653x; 1.0140x over previous
"""Trainium2 Bass kernel for nn_MultiHeadAttention_63814624084186.

Reference computation (per batch sample b, fully independent across b):
  x: [512, 4096]  (C channels x N=64*64 pixels)
  qkv = w_qkv @ x            -> q,k,v each [512, 4096] (8 heads x 64 dims)
  scores = (q_h @ k_h^T)/8   -> [64, 64] per head   (channel-attention)
  attn = softmax(scores, -1)
  out_h = attn_h @ v_h       -> [64, 4096]
  y = w_out @ out + b_out    -> [512, 4096]
  y = groupnorm(y over all C,N) * gamma + beta

Key algebra (this version): attention is over the CHANNEL dim, so
  scores_h = q_h k_h^T = (w_q G w_k^T)_h   with  G = x x^T  [512,512]
  y = w_out bd(A) w_v x = W_eff x          with  W_eff folded on-chip
q, k, v are never materialized.  Per-batch PE work drops from ~4.5e9
MACs (qkv + v + out-proj) to ~2.4e9 (G + y GEMM + small folds).

Sharding: pure data-parallel over batch: 16 samples / 8 cores = 2 per core.

Pipeline (PE queue order; b0/b1 are the two per-core batches):
  G0 T0 sc0 | G1 | R0 W20 T1 sc1 | y0 | R1 W21 | y1 | tails
softmax(b) runs on DVE under the next long PE phase, so the PE never
waits on it.  GroupNorm: bn_stats per psum block, bias folded into the
cross-partition combine (ones-matmul), apply + writeout overlap y1.
"""

import numpy as np
from contextlib import ExitStack

import concourse.bass as bass
import concourse.tile as tile
from concourse import bacc, mybir
from concourse.bass_utils import run_bass_kernel_spmd
from concourse.masks import make_identity

F32 = mybir.dt.float32
F16 = mybir.dt.float16
AX = mybir.AxisListType
ALU = mybir.AluOpType
ACTF = mybir.ActivationFunctionType

B = 16          # global batch
C = 512         # channels
N = 4096        # pixels (64*64)
HW_SIDE = 64
NCORES = 8
PB = B // NCORES  # batches per core
P = 128
KC = C // P     # 4 channel chunks
NB = 8          # n blocks of 512 (y GEMM)
NT = 16         # xT tiles of 2 n-chunks each (G GEMM)
NS = N // 512   # 8 pixel chunks of 512
NHP = 4         # head pairs
XLOOK = 8       # xT DMA lookahead tiles
EPS = 1e-5


def build_nc():
    nc = bacc.Bacc("TRN2", target_bir_lowering=False, debug=False,
                   num_devices=NCORES)

    # xT[b, t, p, j*512+c] = x[b, c, (2t+j)*128 + p]
    xt_d = nc.declare_dram_parameter("xt", [PB, NT, P, 1024], F16, isOutput=False)
    # x[b, nb, p, k*512+n] = x[b, k*128+p, nb*512+n]
    x_d = nc.declare_dram_parameter("x", [PB, NB, P, KC * 512], F16, isOutput=False)
    wq_d = nc.declare_dram_parameter("wq", [P, KC, C], F16, isOutput=False)   # w_q^T
    wk_d = nc.declare_dram_parameter("wk", [P, KC, C], F16, isOutput=False)   # w_k^T
    wv_d = nc.declare_dram_parameter("wv", [P, KC, C], F16, isOutput=False)   # w_v
    wo_d = nc.declare_dram_parameter("wo", [P, KC, C], F16, isOutput=False)   # w_out^T
    bias_d = nc.declare_dram_parameter("bvec", [P, KC], F32, isOutput=False)
    gamma_d = nc.declare_dram_parameter("gamma", [P, KC], F32, isOutput=False)
    beta_d = nc.declare_dram_parameter("beta", [P, KC], F32, isOutput=False)
    out_d = nc.declare_dram_parameter("out", [PB, C, N], F16, isOutput=True)

    with tile.TileContext(nc) as tc, ExitStack() as ctx:
        consts = ctx.enter_context(tc.tile_pool(name="consts", bufs=1))
        xtpool = ctx.enter_context(tc.tile_pool(name="xtpool", bufs=XLOOK + 2))
        xpool = ctx.enter_context(tc.tile_pool(name="xpool", bufs=3))
        gpool = ctx.enter_context(tc.tile_pool(name="gpool", bufs=2))
        tpool = ctx.enter_context(tc.tile_pool(name="tpool", bufs=2))
        rpool = ctx.enter_context(tc.tile_pool(name="rpool", bufs=2))
        w2pool = ctx.enter_context(tc.tile_pool(name="w2pool", bufs=2))
        ypool = ctx.enter_context(tc.tile_pool(name="ypool", bufs=4))
        attn = ctx.enter_context(tc.tile_pool(name="attn", bufs=8))
        attnt = ctx.enter_context(tc.tile_pool(name="attnt", bufs=4))
        stats = ctx.enter_context(tc.tile_pool(name="stats", bufs=4))
        # psg serves both G (4 full banks) and the scores tiles: a matmul
        # start=True resets the target bank's whole per-partition row, so
        # each head-pair's score accumulator needs its own bank (partition
        # packing 0:64/64:128 within a bank is safe, free-offset packing
        # is NOT).  The pool rotation reuses G's banks once G is copied out.
        psg = ctx.enter_context(tc.tile_pool(name="psg", bufs=4, space="PSUM"))
        psmm = ctx.enter_context(tc.tile_pool(name="psmm", bufs=4, space="PSUM"))

        def load_w(dram):
            t = consts.tile([P, KC, C], F16, tag=f"w_{dram.name}")
            nc.sync.dma_start(out=t, in_=dram[:, :, :])
            return t

        xt_tiles = {}

        def fetch_xt(b, t):
            xt = xtpool.tile([P, 2, 512], F16, tag="xt", name=f"xt_{b}_{t}")
            nc.sync.dma_start(
                out=xt, in_=xt_d[b, t].rearrange("p (j c) -> p j c", j=2))
            xt_tiles[(b, t)] = xt

        # xT tiles first: weight loads are deferred until G0's xT stream is
        # fully issued (weights are only needed from T0 onward), so the
        # first G matmuls are never DMA-supply-gated.
        for t in range(XLOOK):
            fetch_xt(0, t)
        W = {}

        bias_sb = consts.tile([P, KC], F32, tag="bias")
        nc.gpsimd.dma_start(out=bias_sb, in_=bias_d[:, :])
        gamma_sb = consts.tile([P, KC], F32, tag="gamma")
        nc.gpsimd.dma_start(out=gamma_sb, in_=gamma_d[:, :])
        beta_sb = consts.tile([P, KC], F32, tag="beta")
        nc.gpsimd.dma_start(out=beta_sb, in_=beta_d[:, :])

        ident_sb = consts.tile([P, P], F16, tag="ident")
        make_identity(nc, ident_sb)
        eps_sb = consts.tile([1, 1], F32, tag="eps")
        nc.vector.memset(eps_sb, EPS)
        # pre-warm the ln+exp activation table: Ln pulls in the table that
        # also contains Exp, so softmax (exp) and rstd (exp(-0.5*ln(var)))
        # never trigger a mid-kernel 1.3us ACT_TABLE_LOAD.
        warm_sb = consts.tile([1, 1], F32, tag="warm")
        nc.scalar.activation(out=warm_sb, in_=eps_sb, func=ACTF.Ln,
                             bias=1.0, scale=0.0)
        ones_col = consts.tile([P, 1], F32, tag="ones_col")
        nc.vector.memset(ones_col, 1.0)
        ones_row = consts.tile([1, P], F32, tag="ones_row")
        nc.vector.memset(ones_row, 1.0)

        # per-batch state carried between emission stages
        st_g = {}    # G in SBUF (f16) [P, KC, C]
        st_t = {}    # T = G @ wk^T   [P, KC, C]
        st_sc = {}   # scores psum tiles (4x [P, 64], head-pair packed)
        st_at = {}   # block-diag attn tiles
        st_r = {}    # R = bd(A)^T @ wo^T
        st_w2 = {}   # W_effT = wv^T-contract @ R
        st_y = {}
        st_stats = {}
        st_scale = {}

        def emit_G(b, prefetched, hook=None):
            """G = x x^T, t-outer: each xT tile is consumed then retired.
            Only the upper block-triangle is computed (rhs = cols >= m*128);
            the 6 lower [128,128] blocks are PE-transposed from the upper
            copies.  All 4 chunk accumulators live in 4 psum banks."""
            g_sb = gpool.tile([P, KC, C], F16, tag="g", name=f"g_{b}")
            st_g[b] = g_sb
            ps = [psg.tile([P, C - m * P], F32, tag="psg", name=f"g_{b}_{m}")
                  for m in range(KC)]
            for t in range(NT):
                tf = t + prefetched
                if tf < NT:
                    fetch_xt(b, tf)
                elif b + 1 < PB and tf - NT < NT:
                    fetch_xt(b + 1, tf - NT)
                if hook and t in hook:
                    hook[t]()
                xt = xt_tiles.pop((b, t))
                for j in range(2):
                    for m in range(KC):
                        nc.tensor.matmul(
                            ps[m],
                            lhsT=xt[:, j, m * P:(m + 1) * P],
                            rhs=xt[:, j, m * P:],
                            start=(t == 0 and j == 0),
                            stop=(t == NT - 1 and j == 1),
                            skip_group_check=True)
            for m in range(KC):
                if m % 2 == 0:
                    nc.scalar.copy(out=g_sb[:, m, m * P:], in_=ps[m])
                else:
                    nc.vector.tensor_copy(out=g_sb[:, m, m * P:], in_=ps[m])
            # lower blocks (m, mp<m) = transpose(upper block (mp, m)),
            # ordered so T chunk 0's operands are ready first
            for m, mp in ((1, 0), (2, 0), (3, 0), (2, 1), (3, 1), (3, 2)):
                pst = psmm.tile([P, P], F16, tag="psmm")
                nc.tensor.transpose(
                    pst, g_sb[:, mp, m * P:(m + 1) * P], ident_sb)
                if (m + mp) % 2 == 0:
                    nc.vector.tensor_copy(
                        out=g_sb[:, m, mp * P:(mp + 1) * P], in_=pst)
                else:
                    nc.scalar.copy(
                        out=g_sb[:, m, mp * P:(mp + 1) * P], in_=pst)

        def emit_T(b):
            """T = G @ wk^T  [c, e], m-outer single-bank accumulation."""
            g_sb = st_g[b]
            t_sb = tpool.tile([P, KC, C], F16, tag="t", name=f"t_{b}")
            st_t[b] = t_sb
            for m in range(KC):
                ps = psmm.tile([P, C], F32, tag="psmm")
                for mp in range(KC):
                    nc.tensor.matmul(
                        ps,
                        lhsT=g_sb[:, mp, m * P:(m + 1) * P],
                        rhs=W['wk'][:, mp, :],
                        start=(mp == 0), stop=(mp == KC - 1))
                nc.vector.tensor_copy(out=t_sb[:, m, :], in_=ps)

        def emit_scores(b):
            """scores_h = (w_q T)_h, two heads packed per psum tile; k-outer
            so the first matmuls only need T chunk 0."""
            t_sb = st_t[b]
            sc_ps = [psg.tile([P, 64], F32, tag="psg", name=f"sc_{b}_{hp}")
                     for hp in range(NHP)]
            st_sc[b] = sc_ps
            for k in range(KC):
                for hp in range(NHP):
                    hA, hB = 2 * hp, 2 * hp + 1
                    clA = slice(hA * 64, hA * 64 + 64)
                    clB = slice(hB * 64, hB * 64 + 64)
                    nc.tensor.matmul(
                        sc_ps[hp][0:64, :],
                        lhsT=W['wq'][:, k, clA], rhs=t_sb[:, k, clA],
                        start=(k == 0), stop=(k == KC - 1),
                        skip_group_check=True)
                    nc.tensor.matmul(
                        sc_ps[hp][64:P, :],
                        lhsT=W['wq'][:, k, clB], rhs=t_sb[:, k, clB],
                        start=(k == 0), stop=(k == KC - 1),
                        skip_group_check=True)

        def emit_softmax(b):
            """softmax over scores (all head pairs batched) -> blockdiag tiles."""
            sc_ps = st_sc[b]
            a_all = attn.tile([P, NHP, 64], F32, tag="a_all")
            for hp in range(NHP):
                nc.vector.tensor_copy(out=a_all[:, hp, :], in_=sc_ps[hp])
            mx = attn.tile([P, NHP, 1], F32, tag="mx4")
            nc.vector.reduce_max(out=mx, in_=a_all, axis=AX.X)
            d_all = attn.tile([P, NHP, 64], F32, tag="d_all")
            nc.vector.tensor_tensor(d_all, a_all,
                                    mx.to_broadcast([P, NHP, 64]), ALU.subtract)
            e_all = attn.tile([P, NHP, 64], F32, tag="e_all")
            nc.scalar.activation(out=e_all, in_=d_all, func=ACTF.Exp,
                                 bias=0.0, scale=0.125)
            sm = attn.tile([P, NHP, 1], F32, tag="sm4")
            nc.vector.reduce_sum(out=sm, in_=e_all, axis=AX.X)
            rs = attn.tile([P, NHP, 1], F32, tag="rs4")
            nc.vector.reciprocal(out=rs, in_=sm)
            a_mm = attn.tile([P, NHP, 64], F16, tag="amm4")
            nc.vector.tensor_tensor(a_mm, e_all,
                                    rs.to_broadcast([P, NHP, 64]), ALU.mult)
            bd_tiles = []
            for hp in range(NHP):
                at = attnt.tile([P, P], F16, tag="attnT", name=f"at_{b}_{hp}")
                nc.gpsimd.memset(at, 0.0)
                nc.vector.tensor_copy(out=at[0:64, 0:64], in_=a_mm[0:64, hp, :])
                nc.vector.tensor_copy(out=at[64:P, 64:P], in_=a_mm[64:P, hp, :])
                bd_tiles.append(at)
            st_at[b] = bd_tiles

        def emit_R(b):
            """R[e, o] = sum_d bd(A)[d, e] wo^T[d, o]."""
            bd_tiles = st_at[b]
            r_sb = rpool.tile([P, KC, C], F16, tag="r", name=f"r_{b}")
            st_r[b] = r_sb
            for hp in range(NHP):
                ps = psmm.tile([P, C], F32, tag="psmm")
                nc.tensor.matmul(ps, lhsT=bd_tiles[hp], rhs=W['wo'][:, hp, :],
                                 start=True, stop=True)
                if hp % 2 == 0:
                    nc.scalar.copy(out=r_sb[:, hp, :], in_=ps)
                else:
                    nc.vector.tensor_copy(out=r_sb[:, hp, :], in_=ps)

        def emit_W2(b):
            """W_effT[c, o] = sum_e wv[e, c] R[e, o], m-outer."""
            r_sb = st_r[b]
            w2 = w2pool.tile([P, KC, C], F16, tag="w2", name=f"w2_{b}")
            st_w2[b] = w2
            for m in range(KC):
                ps = psmm.tile([P, C], F32, tag="psmm")
                for ki in range(KC):
                    nc.tensor.matmul(
                        ps,
                        lhsT=W['wv'][:, ki, m * P:(m + 1) * P],
                        rhs=r_sb[:, ki, :],
                        start=(ki == 0), stop=(ki == KC - 1))
                if m % 2 == 0:
                    nc.scalar.copy(out=w2[:, m, :], in_=ps)
                else:
                    nc.vector.tensor_copy(out=w2[:, m, :], in_=ps)

        st_by = {}

        def emit_By_setup(b):
            y_lo = ypool.tile([P, 2, N], F16, tag="y", name=f"ylo_{b}")
            y_hi = ypool.tile([P, 2, N], F16, tag="y", name=f"yhi_{b}")
            st = stats.tile([P, KC, NS, 6], F32, tag="bnstats")
            mv_t = stats.tile([P, KC, 2], F32, tag="mv")
            st_y[b] = (y_lo, y_hi)
            st_stats[b] = mv_t
            st_by[b] = (st, {})
            for ns in range(2):
                xb = xpool.tile([P, KC, 512], F16, tag="xblk",
                                name=f"x_{b}_{ns}")
                nc.sync.dma_start(
                    out=xb, in_=x_d[b, ns].rearrange("p (k n) -> p k n", k=KC))
                st_by[b][1][ns] = xb

        def emit_By_blocks(b, blocks):
            """y = W_eff @ x (+bias) + bn_stats, streaming x blocks."""
            w2 = st_w2[b]
            y_lo, y_hi = st_y[b]
            st, x_blks = st_by[b]
            for ns in blocks:
                if ns + 2 < NS:
                    xb = xpool.tile([P, KC, 512], F16, tag="xblk",
                                    name=f"x_{b}_{ns + 2}")
                    nc.sync.dma_start(
                        out=xb,
                        in_=x_d[b, ns + 2].rearrange("p (k n) -> p k n", k=KC))
                    x_blks[ns + 2] = xb
                x_blk = x_blks.pop(ns)
                for m in range(KC):
                    yt = y_lo if m < 2 else y_hi
                    mi = m % 2
                    ps = psmm.tile([P, 512], F32, tag="psmm")
                    for k in range(KC):
                        nc.tensor.matmul(
                            ps,
                            lhsT=w2[:, k, m * P:(m + 1) * P],
                            rhs=x_blk[:, k, :],
                            start=(k == 0), stop=(k == KC - 1))
                    # pure-copy psum evacuation (bias folded into the apply
                    # offset) so the psum slot only waits on one engine;
                    # round-robin engines to keep each under the PE rate.
                    ysl = yt[:, mi, ns * 512:(ns + 1) * 512]
                    if m == 1:
                        nc.vector.tensor_copy(out=ysl, in_=ps)
                    else:
                        nc.scalar.copy(out=ysl, in_=ps)
                    # stats read the SBUF copy: they can lag without
                    # backpressuring the psum pool (y is pre-bias)
                    nc.vector.bn_stats(out=st[:, m, ns, :], in_=ysl)

        def emit_By_aggr(b):
            st, _ = st_by[b]
            mv_t = st_stats[b]
            for m in range(KC):
                nc.vector.bn_aggr(out=mv_t[:, m, :], in_=st[:, m])

        def emit_tail_stats(b):
            """global mean/var combine."""
            mv = st_stats[b]
            # S[p, stat, m]: 0 = mean+bias, 1 = var, 2 = (mean+bias)^2
            s_t = stats.tile([P, 3, KC], F32, tag="s_t")
            nc.vector.tensor_add(s_t[:, 0, :], mv[:, :, 0], bias_sb)
            nc.vector.tensor_copy(out=s_t[:, 1, :], in_=mv[:, :, 1])
            nc.vector.tensor_mul(s_t[:, 2, :], s_t[:, 0, :], s_t[:, 0, :])
            pstat = psmm.tile([1, 3, KC], F32, tag="psmm")
            nc.tensor.matmul(pstat, lhsT=ones_col, rhs=s_t,
                             start=True, stop=True)
            red = stats.tile([1, 3], F32, tag="red")
            nc.vector.reduce_sum(out=red, in_=pstat, axis=AX.X)
            e3 = stats.tile([1, 3], F32, tag="e3")
            nc.vector.tensor_scalar_mul(e3, red, 1.0 / C)
            m2 = stats.tile([1, 1], F32, tag="m2")
            nc.vector.tensor_mul(m2, e3[:, 0:1], e3[:, 0:1])
            var = stats.tile([1, 1], F32, tag="var")
            nc.vector.tensor_add(var, e3[:, 1:2], e3[:, 2:3])
            nc.vector.tensor_sub(var, var, m2)
            sc2 = stats.tile([1, 2], F32, tag="sc2")
            nc.vector.tensor_copy(out=sc2[:, 0:1], in_=e3[:, 0:1])
            # rstd = exp(-0.5 * ln(var + eps)): ln and exp share one ACT
            # table (pre-warmed), avoiding the sqrt table swap
            lnv = stats.tile([1, 1], F32, tag="lnv")
            nc.scalar.activation(out=lnv, in_=var, func=ACTF.Ln,
                                 bias=eps_sb, scale=1.0)
            nc.scalar.activation(out=sc2[:, 1:2], in_=lnv, func=ACTF.Exp,
                                 bias=0.0, scale=-0.5)
            bc_ps = psmm.tile([P, 2], F32, tag="psmm")
            nc.tensor.matmul(bc_ps, lhsT=ones_row, rhs=sc2,
                             start=True, stop=True)
            # s = gamma * rstd ; t = beta + (bias - mean_total) * s
            # (bias folded here so the y psum evacuation is a pure copy)
            s_ch = stats.tile([P, KC], F32, tag="s_ch")
            nc.vector.tensor_scalar_mul(s_ch, gamma_sb, bc_ps[:, 1:2])
            t_ch = stats.tile([P, KC], F32, tag="t_ch")
            nc.vector.tensor_scalar_mul(t_ch, s_ch, bc_ps[:, 0:1])
            nc.vector.tensor_sub(t_ch, beta_sb, t_ch)
            tb = stats.tile([P, KC], F32, tag="tb")
            nc.vector.tensor_mul(tb, bias_sb, s_ch)
            nc.vector.tensor_add(t_ch, t_ch, tb)
            st_scale[b] = (s_ch, t_ch)

        def emit_tail_apply(b):
            """normalization apply + writeout."""
            y_lo, y_hi = st_y[b]
            s_ch, t_ch = st_scale[b]
            for m in range(KC):
                yt = y_lo if m < 2 else y_hi
                mi = m % 2
                for h in range(2):
                    sl = slice(h * (N // 2), (h + 1) * (N // 2))
                    if m % 2 == 0:
                        nc.vector.tensor_scalar(
                            out=yt[:, mi, sl], in0=yt[:, mi, sl],
                            scalar1=s_ch[:, m:m + 1], scalar2=t_ch[:, m:m + 1],
                            op0=ALU.mult, op1=ALU.add)
                    else:
                        nc.scalar.activation(
                            out=yt[:, mi, sl], in_=yt[:, mi, sl],
                            func=ACTF.Identity,
                            bias=t_ch[:, m:m + 1], scale=s_ch[:, m:m + 1])
                    nc.sync.dma_start(out=out_d[b, m * P:(m + 1) * P, sl],
                                      in_=yt[:, mi, sl])

        # ---- emission schedule (PE queue order is emission order) ----
        emit_G(0, prefetched=XLOOK, hook={
            NT - 1 - XLOOK: lambda: W.update(wk=load_w(wk_d),
                                             wq=load_w(wq_d))})
        W.update(wo=load_w(wo_d), wv=load_w(wv_d))
        emit_T(0)
        emit_scores(0)
        emit_softmax(0)       # DVE, overlaps G1 on PE
        emit_G(1, prefetched=XLOOK)
        emit_R(0)
        emit_W2(0)
        emit_T(1)
        emit_scores(1)
        emit_softmax(1)       # DVE, overlaps y0 on PE
        emit_By_setup(0)
        emit_By_blocks(0, range(NS))
        emit_By_aggr(0)
        emit_R(1)
        emit_W2(1)
        emit_By_setup(1)
        emit_By_blocks(1, range(2))
        emit_tail_stats(0)    # stat-combine chain hides under By1 blocks
        emit_tail_apply(0)    # DVE/ACT + DMA, overlaps y1 on PE
        emit_By_blocks(1, range(2, NS))
        emit_By_aggr(1)
        emit_tail_stats(1)
        emit_tail_apply(1)

    nc.finalize()
    return nc


_NC_CACHE = {}


def _get_nc():
    if "nc" not in _NC_CACHE:
        _NC_CACHE["nc"] = build_nc()
    return _NC_CACHE["nc"]


def _prep_w(w):
    # [C_in, C_out] -> [128, KC, C_out] fp16 with c_in = k*128 + p
    return np.ascontiguousarray(
        w.reshape(KC, P, C).transpose(1, 0, 2).astype(np.float16))


def _prep_vec(v):
    # [C] -> [128, KC] with c = k*128 + p
    return np.ascontiguousarray(v.reshape(KC, P).T)


def _prep_x(x):
    # [B, C, N] -> [B, NB, P, KC*512] fp16: block j, partition p, (k, n)
    nb = x.shape[0]
    xr = x.reshape(nb, KC, P, NB, 512)
    return np.ascontiguousarray(
        xr.transpose(0, 3, 2, 1, 4).astype(np.float16)).reshape(
        nb, NB, P, KC * 512)


def _prep_xt(x):
    # [B, C, N] -> [B, NT, P, 2*512] fp16: xt[b,t,p,j*512+c] = x[b,c,(2t+j)*128+p]
    nb = x.shape[0]
    xr = x.reshape(nb, C, NT, 2, P)           # [b, c, t, j, p]
    return np.ascontiguousarray(
        xr.transpose(0, 2, 4, 3, 1).astype(np.float16)).reshape(
        nb, NT, P, 1024)


def _make_in_maps(x, w_qkv, w_out, b_out, gamma, beta):
    x = np.asarray(x, dtype=np.float32).reshape(B, C, N)
    xr = _prep_x(x)
    xtr = _prep_xt(x)
    w_qkv = np.asarray(w_qkv, dtype=np.float32)
    wq = _prep_w(np.ascontiguousarray(w_qkv[0:C].T))
    wk = _prep_w(np.ascontiguousarray(w_qkv[C:2 * C].T))
    wv = _prep_w(np.ascontiguousarray(w_qkv[2 * C:3 * C]))
    wo = _prep_w(np.ascontiguousarray(np.asarray(w_out, dtype=np.float32).T))
    bvec = _prep_vec(np.asarray(b_out, dtype=np.float32))
    gam = _prep_vec(np.asarray(gamma, dtype=np.float32))
    bet = _prep_vec(np.asarray(beta, dtype=np.float32))
    return [
        dict(x=np.ascontiguousarray(xr[c * PB:(c + 1) * PB]),
             xt=np.ascontiguousarray(xtr[c * PB:(c + 1) * PB]),
             wq=wq, wk=wk, wv=wv, wo=wo,
             bvec=bvec, gamma=gam, beta=bet)
        for c in range(NCORES)
    ]


def _run(inputs, trace=False, trace_kwargs=None):
    nc = _get_nc()
    in_maps = _make_in_maps(**inputs)
    res = run_bass_kernel_spmd(nc, in_maps, core_ids=list(range(NCORES)),
                               trace=trace, **(trace_kwargs or {}))
    out = np.concatenate([res.results[c]["out"].astype(np.float32)
                          for c in range(NCORES)], axis=0)
    return out.reshape(B, C, HW_SIDE, HW_SIDE), res


def kernel(x, w_qkv, w_out, b_out, gamma, beta):
    inputs = dict(x=x, w_qkv=w_qkv, w_out=w_out, b_out=b_out,
                  gamma=gamma, beta=beta)
    try:
        out, _ = _run(inputs)
    except Exception:
        # transient device errors (e.g. NRT_EXEC_UNIT_UNRECOVERABLE) have
        # been observed once across many runs; one retry recovers.
        out, _ = _run(inputs)
    return out
